# revision 25
# baseline (speedup 1.0000x reference)
"""Devign GGNN model on 8 Trainium2 NeuronCores.

Strategy (data-parallel over graphs, 4 graphs/core):
- Edge gather + scatter-add replaced by dense per-(graph, edge-type)
  adjacency matmuls: a = sum_t A_t @ (h @ W_t.T). A_t is built host-side
  from the integer edge lists (small exact counts, fp8-e4m3).
- Nodes packed (4x513 = 2052 rows, padded to 17x128 = 2176); each graph's
  adjacency strip touches exactly 5 source chunks (513*g starts at chunk 4g).
- fp8-e4m3 DoubleRow (2 contraction rows/cycle) on every 2-chunk
  contraction: adjacency (A exact in fp8), messages (h fp8 stationary,
  wmsg fp8 moving), GRU gates (W fp8 stationary, aT/h fp8 moving), and
  conv stage-1 channel pairs. Step-0 message/GRU h-operands stay bf16
  (h0 = [feature|0] is a single 128-chunk: DoubleRow can't pair it, and
  fp8 without DoubleRow runs at bf16 speed anyway).
- All state SBUF-resident in transposed layouts (feature dim on
  partitions): zero on-device transposes. Scalar+Vector split the
  PSUM->SBUF drains; GpSimd writes the fp8 h copies.
"""

import os
import sys

for _p in ("/opt/trn_rl_repo",):
    if os.path.isdir(_p) and _p not in sys.path:
        sys.path.append(_p)

import numpy as np
import ml_dtypes

BF16 = ml_dtypes.bfloat16
F8 = ml_dtypes.float8_e4m3

B, NN, IN, OUT, T, STEPS = 32, 513, 128, 256, 4, 4
CAT = OUT + IN
NCORES = 8
GPC = B // NCORES          # graphs per core = 4
NV = GPC * NN              # valid packed rows per core = 2052
KCH = 17                   # packed row chunks (2176 = 17 x 128)
NP = KCH * 128             # padded packed rows = 2176
SKC = 5                    # src chunks per graph strip (graph g: chunks 4g..4g+4)
SC = SKC * T               # strip chunk count incl types = 20
SL = [(0, 512), (512, 1024), (1024, 1536), (1536, 2048), (2048, NV)]
ASL = [(0, 320), (320, NN)]  # adjacency dst sub-slabs per graph (513 cols)
L1, P1 = NN - 2, 255       # conv1 out len, pool1 out len
L2Y, P2 = P1, 127          # conv2(k=1) len, final pooled len
L2Z = P1 - 1               # convc2(k=2) out len = 254

_prog_cache = {}


def _build_program(flags):
    import concourse.bacc as bacc
    import concourse.mybir as mybir
    import concourse.tile as tile

    has_bmsg, has_gru_b, has_conv_b, has_mlp_b = flags
    f32 = mybir.dt.float32
    bf16 = mybir.dt.bfloat16
    fp8 = mybir.dt.float8e4
    AF = mybir.ActivationFunctionType
    OP = mybir.AluOpType
    DR = mybir.MatmulPerfMode.DoubleRow

    nc = bacc.Bacc("TRN2", target_bir_lowering=False, debug=False,
                   enable_asserts=False, num_devices=NCORES)

    # ---- DRAM I/O (all pre-laid-out host side, partition dim first) ----
    d_feat = nc.dram_tensor("feat", [128, NP], bf16, kind="ExternalInput").ap()
    d_A = nc.dram_tensor("adj", [128, GPC, SC, NN], fp8, kind="ExternalInput").ap()
    d_wmsg0 = nc.dram_tensor("wmsg0", [128, T, OUT], bf16, kind="ExternalInput").ap()
    d_wmsg8 = nc.dram_tensor("wmsg8", [128, 2, T * OUT], fp8, kind="ExternalInput").ap()
    d_wih8 = nc.dram_tensor("wih8", [128, 2, 3 * OUT], fp8, kind="ExternalInput").ap()
    d_whh8 = nc.dram_tensor("whh8", [128, 2, 3 * OUT], fp8, kind="ExternalInput").ap()
    d_whh0 = nc.dram_tensor("whh0", [128, 3 * OUT], bf16, kind="ExternalInput").ap()
    d_c1w = nc.dram_tensor("c1w", [128, 3, 2, 2, 128], fp8, kind="ExternalInput").ap()
    d_c2w = nc.dram_tensor("c2w", [128, 1, 2, 2, 128], bf16, kind="ExternalInput").ap()
    d_cc1w = nc.dram_tensor("cc1w", [128, 3, 2, 3, 128], fp8, kind="ExternalInput").ap()
    d_cc1wf = nc.dram_tensor("cc1wf", [128, 3, 3, 128], bf16, kind="ExternalInput").ap()
    d_cc2w = nc.dram_tensor("cc2w", [128, 2, 3, 3, 128], bf16, kind="ExternalInput").ap()
    d_mlpy = nc.dram_tensor("mlpy", [128, 2], bf16, kind="ExternalInput").ap()
    d_mlpz = nc.dram_tensor("mlpz", [128, 3], bf16, kind="ExternalInput").ap()
    if has_bmsg:
        d_bmsg = nc.dram_tensor("bmsg", [T, OUT], f32, kind="ExternalInput").ap()
        d_indeg = nc.dram_tensor("indeg", [T, NP], f32, kind="ExternalInput").ap()
    if has_gru_b:
        d_gbias = nc.dram_tensor("gbias", [128, 12], f32, kind="ExternalInput").ap()
    if has_conv_b:
        d_cbias = nc.dram_tensor("cbias", [128, 10], f32, kind="ExternalInput").ap()
    if has_mlp_b:
        d_mbias = nc.dram_tensor("mbias", [1, 2], f32, kind="ExternalInput").ap()
    d_out = nc.dram_tensor("out", [GPC], f32, kind="ExternalOutput").ap()

    def mm_acc(nct, ps, pairs):
        n = len(pairs)
        for i, (l, r) in enumerate(pairs):
            nct.tensor.matmul(ps, l, r, start=(i == 0), stop=(i == n - 1))

    with tile.TileContext(nc) as tc:
        from contextlib import ExitStack
        with ExitStack() as ctx:
            cpool = ctx.enter_context(tc.tile_pool(name="const", bufs=1))
            hpool = ctx.enter_context(tc.tile_pool(name="hstate", bufs=1))
            ypool = ctx.enter_context(tc.tile_pool(name="yact", bufs=2))
            zpool = ctx.enter_context(tc.tile_pool(name="zact", bufs=2))
            ps_hw = ctx.enter_context(
                tc.tile_pool(name="pshw", bufs=2, space="PSUM"))

            # ---- persistent tiles ----
            feat = cpool.tile([128, NP], bf16, tag="feat")
            wmsg0 = cpool.tile([128, T, OUT], bf16, tag="wmsg0")
            wmsg8 = cpool.tile([128, 2, T * OUT], fp8, tag="wmsg8")
            wih8 = cpool.tile([128, 2, 3 * OUT], fp8, tag="wih8")
            whh8 = cpool.tile([128, 2, 3 * OUT], fp8, tag="whh8")
            whh0 = cpool.tile([128, 3 * OUT], bf16, tag="whh0")
            c1w = cpool.tile([128, 3, 2, 2, 128], fp8, tag="c1w")
            c2w = cpool.tile([128, 1, 2, 2, 128], bf16, tag="c2w")
            cc1w = cpool.tile([128, 3, 2, 3, 128], fp8, tag="cc1w")
            cc1wf = cpool.tile([128, 3, 3, 128], bf16, tag="cc1wf")
            cc2w = cpool.tile([128, 2, 3, 3, 128], bf16, tag="cc2w")
            mlpy = cpool.tile([128, 2], bf16, tag="mlpy")
            mlpz = cpool.tile([128, 3], bf16, tag="mlpz")
            hT = hpool.tile([128, 2, NP], bf16, tag="hT")
            h8 = hpool.tile([128, 2, NP], fp8, tag="h8")
            out_sb = cpool.tile([1, GPC], f32, tag="outsb")

            nc.sync.dma_start(out=feat[:, :1024], in_=d_feat[:, :1024])
            nc.sync.dma_start(out=wmsg0[:], in_=d_wmsg0[:])

            if has_conv_b:
                cbias = cpool.tile([128, 10], f32, tag="cbias")
                nc.sync.dma_start(out=cbias[:], in_=d_cbias[:])
            if has_mlp_b:
                mbias = cpool.tile([1, 2], f32, tag="mbias")
                nc.sync.dma_start(out=mbias[:], in_=d_mbias[:])

            # h0 = [feature | 0] is consumed in-place at step 0 (no copy);
            # hT/h8 are first written by the step-0 GRU update. Pad cols
            # (2052:) of h8 are read by chunk-16 message matmuls -> zero once.
            nc.vector.memset(h8[:, :, NV:], 0.0)

            # PE warm-up sized to hide inside the ~2.5us initial DMA
            # latency: ~20 cheap N=128 matmuls on a zeroed scratch trip the
            # HAM activity window so step-0 messages run at 2.4 GHz.
            wsc = cpool.tile([128, 128], bf16, tag="wsc")
            nc.vector.memset(wsc[:], 0.0)
            ps_w = ps_hw.tile([128, 512], f32, tag="pshw", name="warm")
            for i in range(20):
                nc.tensor.matmul(ps_w[:, :128], wsc[:], wsc[:],
                                 start=(i == 0), stop=(i == 19))

            # ================= GGNN =================
            with ExitStack() as gctx:
                apool = gctx.enter_context(tc.tile_pool(name="adj", bufs=1))
                hwpool = gctx.enter_context(tc.tile_pool(name="hw", bufs=2))
                atpool = gctx.enter_context(tc.tile_pool(name="aT", bufs=1))
                grupool = gctx.enter_context(tc.tile_pool(name="gru", bufs=3))
                ps_g = gctx.enter_context(
                    tc.tile_pool(name="psg", bufs=6, space="PSUM"))

                A_sb = apool.tile([128, GPC, SC, NN], fp8, tag="A")
                # g0's adjacency jumps the queue ahead of the second feat
                # half: the first hw chunks only need feat cols 0-1023, and
                # the step-0 A(g0) group is the first DMA-arrival stall
                nc.sync.dma_start(out=feat[:, 1024:], in_=d_feat[:, 1024:])
                # halves so each graph's first adjacency matmuls can start
                # as soon as its first 10 strip chunks land
                for g in range(GPC):
                    nc.sync.dma_start(out=A_sb[:, g, :10], in_=d_A[:, g, :10])
                    nc.sync.dma_start(out=A_sb[:, g, 10:], in_=d_A[:, g, 10:])
                nc.sync.dma_start(out=wih8[:], in_=d_wih8[:])
                nc.sync.dma_start(out=whh8[:], in_=d_whh8[:])
                nc.sync.dma_start(out=whh0[:], in_=d_whh0[:])
                nc.sync.dma_start(out=wmsg8[:], in_=d_wmsg8[:])
                nc.sync.dma_start(out=c1w[:], in_=d_c1w[:])
                nc.sync.dma_start(out=c2w[:], in_=d_c2w[:])
                nc.sync.dma_start(out=cc1w[:], in_=d_cc1w[:])
                nc.sync.dma_start(out=cc1wf[:], in_=d_cc1wf[:])
                nc.sync.dma_start(out=cc2w[:], in_=d_cc2w[:])
                nc.sync.dma_start(out=mlpy[:], in_=d_mlpy[:])
                nc.sync.dma_start(out=mlpz[:], in_=d_mlpz[:])
                aT8 = atpool.tile([128, 2, NP], fp8, tag="aT8")

                if has_bmsg:
                    bmsg = cpool.tile([T, OUT], f32, tag="bmsg")
                    indeg = cpool.tile([T, NP], f32, tag="indeg")
                    nc.sync.dma_start(out=bmsg[:], in_=d_bmsg[:])
                    nc.sync.dma_start(out=indeg[:], in_=d_indeg[:])
                    bias_a = [cpool.tile([128, NP], f32, tag=f"biasa{m}",
                                         name=f"biasa{m}") for m in range(2)]
                    for m in range(2):
                        for (s0, s1) in SL[:4] + [(2048, NP)]:
                            ps = ps_g.tile([128, 512], f32, tag="psg",
                                           name="psb")[:, :s1 - s0]
                            nc.tensor.matmul(
                                ps[:], bmsg[:, m * 128:(m + 1) * 128],
                                indeg[:, s0:s1], start=True, stop=True)
                            nc.vector.tensor_copy(
                                out=bias_a[m][:, s0:s1], in_=ps[:])
                if has_gru_b:
                    gbias = cpool.tile([128, 12], f32, tag="gbias")
                    nc.sync.dma_start(out=gbias[:], in_=d_gbias[:])
                    bias_rz = cpool.tile([128, 4], f32, tag="biasrz")
                    nc.vector.tensor_add(
                        out=bias_rz[:], in0=gbias[:, 0:4], in1=gbias[:, 6:10])

                def msg_phase(s, hw, rc0, rc1):
                    # messages for packed chunks [rc0, rc1) x 2 type-pairs.
                    # step 0: bf16 single-chunk (h0 = [feat|0]);
                    # steps>=1: one fp8 DoubleRow matmul per (chunk, tp).
                    for rc in range(rc0, rc1):
                        for tp in range(2):
                            ps = ps_hw.tile([128, 512], f32, tag="pshw")
                            if s == 0:
                                nc.tensor.matmul(
                                    ps[:], feat[:, rc * 128:(rc + 1) * 128],
                                    wmsg0[:, 2 * tp:2 * tp + 2, :],
                                    start=True, stop=True)
                            else:
                                nc.tensor.matmul(
                                    ps[:], h8[:, :, rc * 128:(rc + 1) * 128],
                                    wmsg8[:, :, tp * 512:(tp + 1) * 512],
                                    start=True, stop=True, perf_mode=DR)
                            hsl = slice(rc * T + 2 * tp, rc * T + 2 * tp + 2)
                            # split PSUM drains between ACT and DVE
                            if rc % 2 == 0:
                                nc.scalar.copy(out=hw[:, hsl, :], in_=ps[:])
                            else:
                                nc.vector.tensor_copy(out=hw[:, hsl, :], in_=ps[:])

                # --- adjacency matmul per graph strip ---
                if True:
                    def a_phase(g, hw):
                        base = g * NN
                        for m in range(2):
                            pa = [ps_g.tile([128, n1 - n0], f32, tag="psg",
                                            name=f"pa{n0}")
                                  for (n0, n1) in ASL]
                            for ps, (n0, n1) in zip(pa, ASL):
                                for i2 in range(SC // 2):
                                    nc.tensor.matmul(
                                        ps[:],
                                        hw[:, 16 * g + 2 * i2:
                                           16 * g + 2 * i2 + 2,
                                           m * 128:(m + 1) * 128],
                                        A_sb[:, g, 2 * i2: 2 * i2 + 2, n0:n1],
                                        start=(i2 == 0), stop=(i2 == SC // 2 - 1),
                                        perf_mode=DR)
                            for ps, (n0, n1) in zip(pa, ASL):
                                if has_bmsg:
                                    nc.vector.tensor_add(
                                        out=aT8[:, m, base + n0:base + n1],
                                        in0=ps[:],
                                        in1=bias_a[m][:, base + n0:base + n1])
                                elif m == 0:
                                    nc.scalar.copy(
                                        out=aT8[:, m, base + n0:base + n1],
                                        in_=ps[:])
                                else:
                                    nc.vector.tensor_copy(
                                        out=aT8[:, m, base + n0:base + n1],
                                        in_=ps[:])

                    # --- GRU, per row slab ---
                    def gru_slab(s, s0, s1):
                        w = s1 - s0
                        cs = slice(s0, s1)
                        rz = grupool.tile([128, 4, 512], bf16, tag="rz",
                                          name="rz")[:, :, :w]
                        nt = grupool.tile([128, 2, 512], bf16, tag="nt",
                                          name="nt")[:, :, :w]
                        for gc in range(4):
                            pp = ps_g.tile([128, 512], f32, tag="psg",
                                           name="pp")[:, :w]
                            nc.tensor.matmul(
                                pp[:], wih8[:, :, gc * 128:(gc + 1) * 128],
                                aT8[:, :, cs], start=True, stop=False,
                                perf_mode=DR)
                            if s == 0:
                                nc.tensor.matmul(
                                    pp[:], whh0[:, gc * 128:(gc + 1) * 128],
                                    feat[:, cs], start=False, stop=True)
                            else:
                                nc.tensor.matmul(
                                    pp[:], whh8[:, :, gc * 128:(gc + 1) * 128],
                                    h8[:, :, cs], start=False, stop=True,
                                    perf_mode=DR)
                            nc.scalar.activation(
                                rz[:, gc, :], pp[:], AF.Sigmoid,
                                bias=bias_rz[:, gc:gc + 1] if has_gru_b else 0.0)
                        for j in range(2):
                            gc = 4 + j
                            pi = ps_g.tile([128, 512], f32, tag="psg",
                                           name="pgi")[:, :w]
                            nc.tensor.matmul(
                                pi[:], wih8[:, :, gc * 128:(gc + 1) * 128],
                                aT8[:, :, cs], start=True, stop=True,
                                perf_mode=DR)
                            ph = ps_g.tile([128, 512], f32, tag="psg",
                                           name="pgh")[:, :w]
                            if s == 0:
                                nc.tensor.matmul(
                                    ph[:], whh0[:, gc * 128:(gc + 1) * 128],
                                    feat[:, cs], start=True, stop=True)
                            else:
                                nc.tensor.matmul(
                                    ph[:], whh8[:, :, gc * 128:(gc + 1) * 128],
                                    h8[:, :, cs], start=True, stop=True,
                                    perf_mode=DR)
                            if has_gru_b:
                                nc.vector.tensor_scalar_add(
                                    out=pi[:], in0=pi[:], scalar1=gbias[:, gc:gc + 1])
                                nc.vector.tensor_scalar_add(
                                    out=ph[:], in0=ph[:], scalar1=gbias[:, 6 + gc:7 + gc])
                            rhn = grupool.tile([128, 512], f32, tag="rhn",
                                               name="rhn")[:, :w]
                            nc.vector.tensor_tensor(
                                out=rhn[:], in0=rz[:, j, :], in1=ph[:], op=OP.mult)
                            nc.vector.tensor_add(out=pi[:], in0=pi[:], in1=rhn[:])
                            nc.scalar.activation(nt[:, j, :], pi[:], AF.Tanh)
                        if s == 0:
                            for m in range(2):
                                d = grupool.tile([128, 2, 512], bf16, tag="d",
                                                 name="d")[:, m, :w]
                                if m == 1:
                                    # h=0: h' = n - z*n
                                    nc.vector.tensor_tensor(
                                        out=d[:], in0=rz[:, 3, :], in1=nt[:, 1, :],
                                        op=OP.mult)
                                    nc.vector.tensor_sub(
                                        out=hT[:, 1, cs], in0=nt[:, 1, :], in1=d[:])
                                    continue
                                nc.vector.tensor_sub(
                                    out=d[:], in0=feat[:, cs], in1=nt[:, 0, :])
                                nc.vector.tensor_tensor(
                                    out=d[:], in0=rz[:, 2, :], in1=d[:], op=OP.mult)
                                nc.vector.tensor_add(
                                    out=hT[:, 0, cs], in0=nt[:, 0, :], in1=d[:])
                            nc.vector.tensor_copy(
                                out=h8[:, :, cs], in_=hT[:, :, cs])
                            return
                        # steps>=1: both m-halves in one [128, 2, w] op each
                        d = grupool.tile([128, 2, 512], bf16, tag="d",
                                         name="d")[:, :, :w]
                        nc.vector.tensor_sub(
                            out=d[:], in0=hT[:, :, cs], in1=nt[:])
                        nc.vector.tensor_tensor(
                            out=d[:], in0=rz[:, 2:4, :], in1=d[:], op=OP.mult)
                        if s < STEPS - 1:
                            # GpSimd deliberately unused: its slow SBUF ops
                            # (~4us per copy) contend for SBUF and slow
                            # concurrent DVE ops 3-6x
                            nc.vector.tensor_add(
                                out=hT[:, :, cs], in0=nt[:], in1=d[:])
                            nc.vector.tensor_add(
                                out=h8[:, :, cs], in0=nt[:], in1=d[:])
                        else:
                            # last step: only the conv head consumes h -> fp8 only
                            nc.vector.tensor_add(
                                out=h8[:, :, cs], in0=nt[:], in1=d[:])

                    # Schedule: within a step, GRU slab k only needs graphs
                    # <= k+1, so its DVE/ACT tail overlaps later graphs'
                    # adjacency mms. Across steps, messages(s+1) for chunks
                    # 0-15 only need h8 cols < 2048 (slabs 0-3), so they run
                    # on the PE while the last slabs' gate chains drain --
                    # removing the step-boundary PE stall.
                    hw_t = [None] * STEPS
                    hw_t[0] = hwpool.tile([128, KCH * T, 256], fp8, tag="hw",
                                          name="hw0")
                    msg_phase(0, hw_t[0], 0, KCH)
                    for s in range(STEPS):
                        hw = hw_t[s]
                        a_phase(0, hw)
                        a_phase(1, hw)
                        gru_slab(s, *SL[0])
                        a_phase(2, hw)
                        gru_slab(s, *SL[1])
                        a_phase(3, hw)
                        gru_slab(s, *SL[2])
                        gru_slab(s, *SL[3])
                        if s < STEPS - 1:
                            hw_t[s + 1] = hwpool.tile(
                                [128, KCH * T, 256], fp8, tag="hw",
                                name=f"hw{s + 1}")
                            msg_phase(s + 1, hw_t[s + 1], 0, KCH - 1)
                        gru_slab(s, *SL[4])
                        if s < STEPS - 1:
                            msg_phase(s + 1, hw_t[s + 1], KCH - 1, KCH)
            # ================= conv heads =================
            # ps_hw pool is still live (outer scope) on banks disjoint from
            # the GGNN gate pools, so conv matmuls don't serialize on the
            # GGNN drain at the phase transition
            if True:

                def conv_stage1(g):
                    base = g * NN
                    # stage 1: all five conv1 output chunks (Y then Z) so PE
                    # has a long uninterrupted run while pools/relu trail
                    y1 = ypool.tile([128, 2, L1], bf16, tag="y1")
                    z1 = zpool.tile([128, 3, L1], bf16, tag="z1")
                    for co in range(2):
                        ps = ps_hw.tile([128, L1], f32, tag="pshw")
                        for k in range(3):
                            nc.tensor.matmul(
                                ps[:], c1w[:, k, :, co, :],
                                h8[:, :, base + k: base + k + L1],
                                start=(k == 0), stop=(k == 2), perf_mode=DR)
                        nc.scalar.activation(
                            y1[:, co, :], ps[:], AF.Relu,
                            bias=cbias[:, co:co + 1] if has_conv_b else 0.0)
                    for co in range(3):
                        ps = ps_hw.tile([128, L1], f32, tag="pshw")
                        for k in range(3):
                            nc.tensor.matmul(
                                ps[:], cc1w[:, k, :, co, :],
                                h8[:, :, base + k: base + k + L1],
                                start=(k == 0), stop=False, perf_mode=DR)
                            nc.tensor.matmul(
                                ps[:], cc1wf[:, k, co, :],
                                feat[:, base + k: base + k + L1],
                                start=False, stop=(k == 2))
                        nc.scalar.activation(
                            z1[:, co, :], ps[:], AF.Relu,
                            bias=cbias[:, 4 + co:5 + co] if has_conv_b else 0.0)
                    # stage 2: pools
                    y1p = ypool.tile([128, 2, P1], bf16, tag="y1p")
                    z1p = zpool.tile([128, 3, P1], bf16, tag="z1p")
                    for co in range(2):
                        nc.vector.tensor_tensor(
                            out=y1p[:, co, :], in0=y1[:, co, 0:510:2],
                            in1=y1[:, co, 1:510:2], op=OP.max)
                        nc.vector.tensor_tensor(
                            out=y1p[:, co, :], in0=y1p[:, co, :],
                            in1=y1[:, co, 2:511:2], op=OP.max)
                    for co in range(3):
                        nc.vector.tensor_tensor(
                            out=z1p[:, co, :], in0=z1[:, co, 0:510:2],
                            in1=z1[:, co, 1:510:2], op=OP.max)
                        nc.vector.tensor_tensor(
                            out=z1p[:, co, :], in0=z1p[:, co, :],
                            in1=z1[:, co, 2:511:2], op=OP.max)
                    return y1p, z1p

                def conv_rest(g, y1p, z1p):
                    # stage 3: second convs (bf16: short free dims are
                    # LDWEIGHTS-bound, DoubleRow wouldn't help)
                    y2 = ypool.tile([128, 2, L2Y], bf16, tag="y2")
                    z2 = zpool.tile([128, 3, L2Z], bf16, tag="z2")
                    for co in range(2):
                        ps = ps_hw.tile([128, L2Y], f32, tag="pshw")
                        mm_acc(nc, ps[:], [
                            (c2w[:, 0, ci, co, :], y1p[:, ci, :]) for ci in range(2)])
                        nc.scalar.activation(
                            y2[:, co, :], ps[:], AF.Relu,
                            bias=cbias[:, 2 + co:3 + co] if has_conv_b else 0.0)
                    for co in range(3):
                        ps = ps_hw.tile([128, L2Z], f32, tag="pshw")
                        mm_acc(nc, ps[:], [
                            (cc2w[:, k, ci, co, :], z1p[:, ci, k:k + L2Z])
                            for k in range(2) for ci in range(3)])
                        nc.scalar.activation(
                            z2[:, co, :], ps[:], AF.Relu,
                            bias=cbias[:, 7 + co:8 + co] if has_conv_b else 0.0)
                    # stage 4: pools + heads
                    y2p = ypool.tile([128, 2, P2], bf16, tag="y2p")
                    z2p = zpool.tile([128, 3, P2], bf16, tag="z2p")
                    for co in range(2):
                        nc.vector.tensor_tensor(
                            out=y2p[:, co, :], in0=y2[:, co, 0:254:2],
                            in1=y2[:, co, 1:254:2], op=OP.max)
                    for co in range(3):
                        nc.vector.tensor_tensor(
                            out=z2p[:, co, :], in0=z2[:, co, 0:254:2],
                            in1=z2[:, co, 1:254:2], op=OP.max)
                    psy = ps_hw.tile([1, P2], f32, tag="pshw")
                    mm_acc(nc, psy[:], [
                        (mlpy[:, co:co + 1], y2p[:, co, :]) for co in range(2)])
                    ys = ypool.tile([1, P2], f32, tag="ys")
                    if has_mlp_b:
                        nc.vector.tensor_scalar_add(
                            out=ys[:], in0=psy[:], scalar1=mbias[:, 0:1])
                    else:
                        nc.vector.tensor_copy(out=ys[:], in_=psy[:])
                    psz = ps_hw.tile([1, P2], f32, tag="pshw")
                    mm_acc(nc, psz[:], [
                        (mlpz[:, co:co + 1], z2p[:, co, :]) for co in range(3)])
                    zs = zpool.tile([1, P2], f32, tag="zs")
                    if has_mlp_b:
                        nc.vector.tensor_scalar_add(
                            out=zs[:], in0=psz[:], scalar1=mbias[:, 1:2])
                    else:
                        nc.vector.tensor_copy(out=zs[:], in_=psz[:])
                    prod = ypool.tile([1, P2], f32, tag="prod")
                    nc.vector.tensor_tensor(
                        out=prod[:], in0=ys[:], in1=zs[:], op=OP.mult)
                    red = ypool.tile([1, 1], f32, tag="red")
                    import concourse.mybir as _mb
                    nc.vector.reduce_sum(red[:], prod[:], axis=_mb.AxisListType.X)
                    nc.scalar.activation(
                        out_sb[:, g:g + 1], red[:], AF.Sigmoid, scale=1.0 / P2)

                # pipeline graphs one deep: stage1(g+1)'s matmuls keep the
                # PE busy while stage3(g) waits on stage-2 pools (DVE)
                pending = conv_stage1(0)
                for g in range(1, GPC):
                    nxt = conv_stage1(g)
                    conv_rest(g - 1, *pending)
                    pending = nxt
                conv_rest(GPC - 1, *pending)

            nc.sync.dma_start(out=d_out[None, :], in_=out_sb[:1, :])

    nc.compile()
    return nc


def _layout_inputs(feature, W_msg, b_msg, gru_w_ih, gru_w_hh, gru_b_ih, gru_b_hh,
                   conv1_w, conv1_b, conv2_w, conv2_b, convc1_w, convc1_b,
                   convc2_w, convc2_b, mlpy_w, mlpy_b, mlpz_w, mlpz_b,
                   edge_src, edge_dst, edge_type):
    """Host-side sharding + SBUF-layout construction. Index math only
    (plus dtype casts / zero padding / transposes of float inputs)."""
    feature = np.asarray(feature, np.float32)
    edge_src = np.asarray(edge_src).astype(np.int64)
    edge_dst = np.asarray(edge_dst).astype(np.int64)
    edge_type = np.asarray(edge_type).astype(np.int64)

    flags = (
        bool(np.any(np.asarray(b_msg))),
        bool(np.any(np.asarray(gru_b_ih)) or np.any(np.asarray(gru_b_hh))),
        bool(np.any(np.asarray(conv1_b)) or np.any(np.asarray(conv2_b))
             or np.any(np.asarray(convc1_b)) or np.any(np.asarray(convc2_b))),
        bool(np.any(np.asarray(mlpy_b)) or np.any(np.asarray(mlpz_b))),
    )
    has_bmsg, has_gru_b, has_conv_b, has_mlp_b = flags

    # ---- adjacency counts, padded to 640/graph ----
    g_of_e = edge_src // NN
    d_loc = edge_dst - g_of_e * NN
    # packed per-core row index of src: 513*(g mod GPC) + s_loc
    src_packed = edge_src - (g_of_e // GPC) * (GPC * NN)
    kc = src_packed // 128           # packed chunk 0..16 (per core)
    pp = src_packed - kc * 128
    kloc = kc - 4 * (g_of_e % GPC)   # strip chunk 0..4
    # A_h[p, core, g, kloc*T+t, d]
    A_h = np.zeros((128, NCORES, GPC, SC, NN), np.float32)
    np.add.at(A_h, (pp, g_of_e // GPC, g_of_e % GPC,
                    kloc * T + edge_type, d_loc), 1.0)

    # ---- shared weight layouts ----
    W_msg = np.asarray(W_msg, np.float32)          # [T, out, in]
    wmsg_l = np.ascontiguousarray(
        W_msg.transpose(2, 0, 1).reshape(2, 128, T, OUT)
        .transpose(1, 0, 2, 3))                     # [p, k, t, o] f32
    wih_l = np.ascontiguousarray(
        np.asarray(gru_w_ih, np.float32).T.reshape(2, 128, 3 * OUT)
        .transpose(1, 0, 2))                        # [p, k, m] f32
    whh_l = np.ascontiguousarray(
        np.asarray(gru_w_hh, np.float32).T.reshape(2, 128, 3 * OUT)
        .transpose(1, 0, 2))

    def conv_lay(w, nci, nco):
        # w: [cout, cin, k] -> [p, k, ci, co, f]; lhsT[cin_part, cout_free]
        w = np.asarray(w, np.float32)
        k = w.shape[2]
        out = np.zeros((128, k, nci, nco, 128), np.float32)
        for kk in range(k):
            wt = w[:, :, kk].T                      # [cin, cout]
            for ci in range(nci):
                for co in range(nco):
                    out[:, kk, ci, co, :] = wt[ci * 128:(ci + 1) * 128,
                                               co * 128:(co + 1) * 128]
        return out

    c1w_l = conv_lay(conv1_w, 2, 2)
    c2w_l = conv_lay(conv2_w, 2, 2).astype(BF16)
    cc1w_l = conv_lay(convc1_w, 3, 3)
    cc2w_l = conv_lay(convc2_w, 3, 3).astype(BF16)
    mlpy_l = np.ascontiguousarray(
        np.asarray(mlpy_w, np.float32).reshape(2, 128).T).astype(BF16)
    mlpz_l = np.ascontiguousarray(
        np.asarray(mlpz_w, np.float32).reshape(3, 128).T).astype(BF16)

    shared = dict(
        wmsg0=np.ascontiguousarray(wmsg_l[:, 0]).astype(BF16),
        wmsg8=np.ascontiguousarray(wmsg_l.reshape(128, 2, T * OUT)).astype(F8),
        wih8=wih_l.astype(F8),
        whh8=whh_l.astype(F8),
        whh0=np.ascontiguousarray(whh_l[:, 0]).astype(BF16),
        c1w=c1w_l.astype(F8),
        c2w=c2w_l,
        cc1w=np.ascontiguousarray(cc1w_l[:, :, 0:2]).astype(F8),
        cc1wf=np.ascontiguousarray(cc1w_l[:, :, 2]).astype(BF16),
        cc2w=cc2w_l, mlpy=mlpy_l, mlpz=mlpz_l)
    if has_bmsg:
        shared["bmsg"] = np.asarray(b_msg, np.float32)
    if has_gru_b:
        gb = np.zeros((128, 12), np.float32)
        gb[:, 0:6] = np.asarray(gru_b_ih, np.float32).reshape(6, 128).T
        gb[:, 6:12] = np.asarray(gru_b_hh, np.float32).reshape(6, 128).T
        shared["gbias"] = gb
    if has_conv_b:
        cb = np.zeros((128, 10), np.float32)
        cb[:, 0:2] = np.asarray(conv1_b, np.float32).reshape(2, 128).T
        cb[:, 2:4] = np.asarray(conv2_b, np.float32).reshape(2, 128).T
        cb[:, 4:7] = np.asarray(convc1_b, np.float32).reshape(3, 128).T
        cb[:, 7:10] = np.asarray(convc2_b, np.float32).reshape(3, 128).T
        shared["cbias"] = cb
    if has_mlp_b:
        shared["mbias"] = np.array(
            [[float(np.asarray(mlpy_b).reshape(-1)[0]),
              float(np.asarray(mlpz_b).reshape(-1)[0])]], np.float32)

    in_maps = []
    for c in range(NCORES):
        g0 = c * GPC
        feat_l = np.zeros((128, NP), np.float32)
        rows = feature[g0 * NN:(g0 + GPC) * NN]                # [2052, 128]
        feat_l[:, :NV] = rows.T
        A_l = np.ascontiguousarray(A_h[:, c]).astype(F8)       # [128,4,20,513]
        m = dict(shared)
        m["feat"] = feat_l.astype(BF16)
        m["adj"] = A_l
        if has_bmsg:
            ind = np.zeros((T, NP), np.float32)
            for g in range(GPC):
                ed_g = (g_of_e // GPC == c) & (g_of_e % GPC == g)
                np.add.at(ind, (edge_type[ed_g], g * NN + d_loc[ed_g]), 1.0)
            m["indeg"] = ind
        in_maps.append(m)
    return flags, in_maps


def kernel(**inputs):
    from concourse.bass_utils import run_bass_kernel_spmd

    flags, in_maps = _layout_inputs(**inputs)
    if flags not in _prog_cache:
        _prog_cache[flags] = _build_program(flags)
    nc = _prog_cache[flags]
    res = run_bass_kernel_spmd(nc, in_maps, core_ids=list(range(NCORES)))
    out = np.concatenate([np.asarray(res.results[c]["out"], np.float32)
                          for c in range(NCORES)])
    return out


# revision 26
# speedup vs baseline: 1.0178x; 1.0178x over previous
"""Devign GGNN model on 8 Trainium2 NeuronCores.

Strategy (data-parallel over graphs, 4 graphs/core):
- Edge gather + scatter-add replaced by dense per-(graph, edge-type)
  adjacency matmuls: a = sum_t A_t @ (h @ W_t.T). A_t is built host-side
  from the integer edge lists (small exact counts, fp8-e4m3).
- Nodes packed (4x513 = 2052 rows, padded to 17x128 = 2176); each graph's
  adjacency strip touches exactly 5 source chunks (513*g starts at chunk 4g).
- fp8-e4m3 DoubleRow (2 contraction rows/cycle) on every 2-chunk
  contraction: adjacency (A exact in fp8), messages (h fp8 stationary,
  wmsg fp8 moving), GRU gates (W fp8 stationary, aT/h fp8 moving), and
  conv stage-1 channel pairs. Step-0 message/GRU h-operands stay bf16
  (h0 = [feature|0] is a single 128-chunk: DoubleRow can't pair it, and
  fp8 without DoubleRow runs at bf16 speed anyway).
- All state SBUF-resident in transposed layouts (feature dim on
  partitions): zero on-device transposes. Scalar+Vector split the
  PSUM->SBUF drains; GpSimd writes the fp8 h copies.
"""

import os
import sys

for _p in ("/opt/trn_rl_repo",):
    if os.path.isdir(_p) and _p not in sys.path:
        sys.path.append(_p)

import numpy as np
import ml_dtypes

BF16 = ml_dtypes.bfloat16
F8 = ml_dtypes.float8_e4m3

B, NN, IN, OUT, T, STEPS = 32, 513, 128, 256, 4, 4
CAT = OUT + IN
NCORES = 8
GPC = B // NCORES          # graphs per core = 4
NV = GPC * NN              # valid packed rows per core = 2052
KCH = 17                   # packed row chunks (2176 = 17 x 128)
NP = KCH * 128             # padded packed rows = 2176
SKC = 5                    # src chunks per graph strip (graph g: chunks 4g..4g+4)
SC = SKC * T               # strip chunk count incl types = 20
SL = [(0, 512), (512, 1024), (1024, 1536), (1536, 2048), (2048, NV)]
ASL = [(0, 320), (320, NN)]  # adjacency dst sub-slabs per graph (513 cols)
L1, P1 = NN - 2, 255       # conv1 out len, pool1 out len
L2Y, P2 = P1, 127          # conv2(k=1) len, final pooled len
L2Z = P1 - 1               # convc2(k=2) out len = 254

_prog_cache = {}


def _build_program(flags):
    import concourse.bacc as bacc
    import concourse.mybir as mybir
    import concourse.tile as tile

    has_bmsg, has_gru_b, has_conv_b, has_mlp_b = flags
    f32 = mybir.dt.float32
    bf16 = mybir.dt.bfloat16
    fp8 = mybir.dt.float8e4
    AF = mybir.ActivationFunctionType
    OP = mybir.AluOpType
    DR = mybir.MatmulPerfMode.DoubleRow

    nc = bacc.Bacc("TRN2", target_bir_lowering=False, debug=False,
                   enable_asserts=False, num_devices=NCORES)

    # ---- DRAM I/O (all pre-laid-out host side, partition dim first) ----
    d_feat = nc.dram_tensor("feat", [128, NP], bf16, kind="ExternalInput").ap()
    d_A = nc.dram_tensor("adj", [128, GPC, SC, NN], fp8, kind="ExternalInput").ap()
    d_wmsg0 = nc.dram_tensor("wmsg0", [128, T, OUT], bf16, kind="ExternalInput").ap()
    d_wmsg8 = nc.dram_tensor("wmsg8", [128, 2, T * OUT], fp8, kind="ExternalInput").ap()
    d_wih8 = nc.dram_tensor("wih8", [128, 2, 3 * OUT], fp8, kind="ExternalInput").ap()
    d_whh8 = nc.dram_tensor("whh8", [128, 2, 3 * OUT], fp8, kind="ExternalInput").ap()
    d_whh0 = nc.dram_tensor("whh0", [128, 3 * OUT], bf16, kind="ExternalInput").ap()
    d_c1w = nc.dram_tensor("c1w", [128, 3, 2, 2, 128], fp8, kind="ExternalInput").ap()
    d_c2w = nc.dram_tensor("c2w", [128, 1, 2, 2, 128], bf16, kind="ExternalInput").ap()
    d_cc1w = nc.dram_tensor("cc1w", [128, 3, 2, 3, 128], fp8, kind="ExternalInput").ap()
    d_cc1wf = nc.dram_tensor("cc1wf", [128, 3, 3, 128], bf16, kind="ExternalInput").ap()
    d_cc2w = nc.dram_tensor("cc2w", [128, 2, 3, 3, 128], bf16, kind="ExternalInput").ap()
    d_mlpy = nc.dram_tensor("mlpy", [128, 2], bf16, kind="ExternalInput").ap()
    d_mlpz = nc.dram_tensor("mlpz", [128, 3], bf16, kind="ExternalInput").ap()
    if has_bmsg:
        d_bmsg = nc.dram_tensor("bmsg", [T, OUT], f32, kind="ExternalInput").ap()
        d_indeg = nc.dram_tensor("indeg", [T, NP], f32, kind="ExternalInput").ap()
    if has_gru_b:
        d_gbias = nc.dram_tensor("gbias", [128, 12], f32, kind="ExternalInput").ap()
    if has_conv_b:
        d_cbias = nc.dram_tensor("cbias", [128, 10], f32, kind="ExternalInput").ap()
    if has_mlp_b:
        d_mbias = nc.dram_tensor("mbias", [1, 2], f32, kind="ExternalInput").ap()
    d_out = nc.dram_tensor("out", [GPC], f32, kind="ExternalOutput").ap()

    def mm_acc(nct, ps, pairs):
        n = len(pairs)
        for i, (l, r) in enumerate(pairs):
            nct.tensor.matmul(ps, l, r, start=(i == 0), stop=(i == n - 1))

    with tile.TileContext(nc) as tc:
        from contextlib import ExitStack
        with ExitStack() as ctx:
            cpool = ctx.enter_context(tc.tile_pool(name="const", bufs=1))
            hpool = ctx.enter_context(tc.tile_pool(name="hstate", bufs=1))
            ypool = ctx.enter_context(tc.tile_pool(name="yact", bufs=2))
            zpool = ctx.enter_context(tc.tile_pool(name="zact", bufs=2))
            ps_hw = ctx.enter_context(
                tc.tile_pool(name="pshw", bufs=2, space="PSUM"))

            # ---- persistent tiles ----
            feat = cpool.tile([128, NP], bf16, tag="feat")
            wmsg0 = cpool.tile([128, T, OUT], bf16, tag="wmsg0")
            wmsg8 = cpool.tile([128, 2, T * OUT], fp8, tag="wmsg8")
            wih8 = cpool.tile([128, 2, 3 * OUT], fp8, tag="wih8")
            whh8 = cpool.tile([128, 2, 3 * OUT], fp8, tag="whh8")
            whh0 = cpool.tile([128, 3 * OUT], bf16, tag="whh0")
            c1w = cpool.tile([128, 3, 2, 2, 128], fp8, tag="c1w")
            c2w = cpool.tile([128, 1, 2, 2, 128], bf16, tag="c2w")
            cc1w = cpool.tile([128, 3, 2, 3, 128], fp8, tag="cc1w")
            cc1wf = cpool.tile([128, 3, 3, 128], bf16, tag="cc1wf")
            cc2w = cpool.tile([128, 2, 3, 3, 128], bf16, tag="cc2w")
            mlpy = cpool.tile([128, 2], bf16, tag="mlpy")
            mlpz = cpool.tile([128, 3], bf16, tag="mlpz")
            hT = hpool.tile([128, 2, NP], bf16, tag="hT")
            h8 = hpool.tile([128, 2, NP], fp8, tag="h8")
            out_sb = cpool.tile([1, GPC], f32, tag="outsb")

            nc.sync.dma_start(out=feat[:, :1024], in_=d_feat[:, :1024])
            nc.sync.dma_start(out=wmsg0[:], in_=d_wmsg0[:])

            if has_conv_b:
                cbias = cpool.tile([128, 10], f32, tag="cbias")
                nc.sync.dma_start(out=cbias[:], in_=d_cbias[:])
            if has_mlp_b:
                mbias = cpool.tile([1, 2], f32, tag="mbias")
                nc.sync.dma_start(out=mbias[:], in_=d_mbias[:])

            # h0 = [feature | 0] is consumed in-place at step 0 (no copy);
            # hT/h8 are first written by the step-0 GRU update. Pad cols
            # (2052:) of h8 are read by chunk-16 message matmuls -> zero once.
            nc.vector.memset(h8[:, :, NV:], 0.0)

            # PE warm-up sized to hide inside the ~2.5us initial DMA
            # latency: ~20 cheap N=128 matmuls on a zeroed scratch trip the
            # HAM activity window so step-0 messages run at 2.4 GHz.
            wsc = cpool.tile([128, 128], bf16, tag="wsc")
            nc.vector.memset(wsc[:], 0.0)
            ps_w = ps_hw.tile([128, 512], f32, tag="pshw", name="warm")
            for i in range(20):
                nc.tensor.matmul(ps_w[:, :128], wsc[:], wsc[:],
                                 start=(i == 0), stop=(i == 19))

            # ================= GGNN =================
            with ExitStack() as gctx:
                apool = gctx.enter_context(tc.tile_pool(name="adj", bufs=1))
                hwpool = gctx.enter_context(tc.tile_pool(name="hw", bufs=2))
                atpool = gctx.enter_context(tc.tile_pool(name="aT", bufs=1))
                grupool = gctx.enter_context(tc.tile_pool(name="gru", bufs=3))
                ps_g = gctx.enter_context(
                    tc.tile_pool(name="psg", bufs=6, space="PSUM"))

                A_sb = apool.tile([128, GPC, SC, NN], fp8, tag="A")
                # g0's adjacency jumps the queue ahead of the second feat
                # half: the first hw chunks only need feat cols 0-1023, and
                # the step-0 A(g0) group is the first DMA-arrival stall
                nc.sync.dma_start(out=feat[:, 1024:], in_=d_feat[:, 1024:])
                # halves so each graph's first adjacency matmuls can start
                # as soon as its first 10 strip chunks land
                for g in range(GPC):
                    nc.sync.dma_start(out=A_sb[:, g, :10], in_=d_A[:, g, :10])
                    nc.sync.dma_start(out=A_sb[:, g, 10:], in_=d_A[:, g, 10:])
                nc.sync.dma_start(out=wih8[:], in_=d_wih8[:])
                nc.sync.dma_start(out=whh8[:], in_=d_whh8[:])
                nc.sync.dma_start(out=whh0[:], in_=d_whh0[:])
                nc.sync.dma_start(out=wmsg8[:], in_=d_wmsg8[:])
                nc.sync.dma_start(out=c1w[:], in_=d_c1w[:])
                nc.sync.dma_start(out=c2w[:], in_=d_c2w[:])
                nc.sync.dma_start(out=cc1w[:], in_=d_cc1w[:])
                nc.sync.dma_start(out=cc1wf[:], in_=d_cc1wf[:])
                nc.sync.dma_start(out=cc2w[:], in_=d_cc2w[:])
                nc.sync.dma_start(out=mlpy[:], in_=d_mlpy[:])
                nc.sync.dma_start(out=mlpz[:], in_=d_mlpz[:])
                aT8 = atpool.tile([128, 2, NP], fp8, tag="aT8")

                if has_bmsg:
                    bmsg = cpool.tile([T, OUT], f32, tag="bmsg")
                    indeg = cpool.tile([T, NP], f32, tag="indeg")
                    nc.sync.dma_start(out=bmsg[:], in_=d_bmsg[:])
                    nc.sync.dma_start(out=indeg[:], in_=d_indeg[:])
                    bias_a = [cpool.tile([128, NP], f32, tag=f"biasa{m}",
                                         name=f"biasa{m}") for m in range(2)]
                    for m in range(2):
                        for (s0, s1) in SL[:4] + [(2048, NP)]:
                            ps = ps_g.tile([128, 512], f32, tag="psg",
                                           name="psb")[:, :s1 - s0]
                            nc.tensor.matmul(
                                ps[:], bmsg[:, m * 128:(m + 1) * 128],
                                indeg[:, s0:s1], start=True, stop=True)
                            nc.vector.tensor_copy(
                                out=bias_a[m][:, s0:s1], in_=ps[:])
                if has_gru_b:
                    gbias = cpool.tile([128, 12], f32, tag="gbias")
                    nc.sync.dma_start(out=gbias[:], in_=d_gbias[:])
                    bias_rz = cpool.tile([128, 4], f32, tag="biasrz")
                    nc.vector.tensor_add(
                        out=bias_rz[:], in0=gbias[:, 0:4], in1=gbias[:, 6:10])

                def msg_phase(s, hw, rc0, rc1):
                    # messages for packed chunks [rc0, rc1) x 2 type-pairs.
                    # step 0: bf16 single-chunk (h0 = [feat|0]);
                    # steps>=1: one fp8 DoubleRow matmul per (chunk, tp).
                    for rc in range(rc0, rc1):
                        for tp in range(2):
                            ps = ps_hw.tile([128, 512], f32, tag="pshw")
                            if s == 0:
                                nc.tensor.matmul(
                                    ps[:], feat[:, rc * 128:(rc + 1) * 128],
                                    wmsg0[:, 2 * tp:2 * tp + 2, :],
                                    start=True, stop=True)
                            else:
                                nc.tensor.matmul(
                                    ps[:], h8[:, :, rc * 128:(rc + 1) * 128],
                                    wmsg8[:, :, tp * 512:(tp + 1) * 512],
                                    start=True, stop=True, perf_mode=DR)
                            hsl = slice(rc * T + 2 * tp, rc * T + 2 * tp + 2)
                            # split PSUM drains between ACT and DVE
                            if rc % 2 == 0:
                                nc.scalar.copy(out=hw[:, hsl, :], in_=ps[:])
                            else:
                                nc.vector.tensor_copy(out=hw[:, hsl, :], in_=ps[:])

                # --- adjacency matmul per graph strip ---
                if True:
                    def a_phase(g, hw):
                        base = g * NN
                        for m in range(2):
                            pa = [ps_g.tile([128, n1 - n0], f32, tag="psg",
                                            name=f"pa{n0}")
                                  for (n0, n1) in ASL]
                            for ps, (n0, n1) in zip(pa, ASL):
                                for i2 in range(SC // 2):
                                    nc.tensor.matmul(
                                        ps[:],
                                        hw[:, 16 * g + 2 * i2:
                                           16 * g + 2 * i2 + 2,
                                           m * 128:(m + 1) * 128],
                                        A_sb[:, g, 2 * i2: 2 * i2 + 2, n0:n1],
                                        start=(i2 == 0), stop=(i2 == SC // 2 - 1),
                                        perf_mode=DR)
                            for ps, (n0, n1) in zip(pa, ASL):
                                if has_bmsg:
                                    nc.vector.tensor_add(
                                        out=aT8[:, m, base + n0:base + n1],
                                        in0=ps[:],
                                        in1=bias_a[m][:, base + n0:base + n1])
                                else:
                                    nc.scalar.copy(
                                        out=aT8[:, m, base + n0:base + n1],
                                        in_=ps[:])

                    # --- GRU, per row slab ---
                    def gru_slab(s, s0, s1):
                        w = s1 - s0
                        cs = slice(s0, s1)
                        rz = grupool.tile([128, 4, 512], bf16, tag="rz",
                                          name="rz")[:, :, :w]
                        nt = grupool.tile([128, 2, 512], bf16, tag="nt",
                                          name="nt")[:, :, :w]
                        for gc in range(4):
                            pp = ps_g.tile([128, 512], f32, tag="psg",
                                           name="pp")[:, :w]
                            nc.tensor.matmul(
                                pp[:], wih8[:, :, gc * 128:(gc + 1) * 128],
                                aT8[:, :, cs], start=True, stop=False,
                                perf_mode=DR)
                            if s == 0:
                                nc.tensor.matmul(
                                    pp[:], whh0[:, gc * 128:(gc + 1) * 128],
                                    feat[:, cs], start=False, stop=True)
                            else:
                                nc.tensor.matmul(
                                    pp[:], whh8[:, :, gc * 128:(gc + 1) * 128],
                                    h8[:, :, cs], start=False, stop=True,
                                    perf_mode=DR)
                            nc.scalar.activation(
                                rz[:, gc, :], pp[:], AF.Sigmoid,
                                bias=bias_rz[:, gc:gc + 1] if has_gru_b else 0.0)
                        for j in range(2):
                            gc = 4 + j
                            pi = ps_g.tile([128, 512], f32, tag="psg",
                                           name="pgi")[:, :w]
                            nc.tensor.matmul(
                                pi[:], wih8[:, :, gc * 128:(gc + 1) * 128],
                                aT8[:, :, cs], start=True, stop=True,
                                perf_mode=DR)
                            ph = ps_g.tile([128, 512], f32, tag="psg",
                                           name="pgh")[:, :w]
                            if s == 0:
                                nc.tensor.matmul(
                                    ph[:], whh0[:, gc * 128:(gc + 1) * 128],
                                    feat[:, cs], start=True, stop=True)
                            else:
                                nc.tensor.matmul(
                                    ph[:], whh8[:, :, gc * 128:(gc + 1) * 128],
                                    h8[:, :, cs], start=True, stop=True,
                                    perf_mode=DR)
                            if has_gru_b:
                                nc.vector.tensor_scalar_add(
                                    out=pi[:], in0=pi[:], scalar1=gbias[:, gc:gc + 1])
                                nc.vector.tensor_scalar_add(
                                    out=ph[:], in0=ph[:], scalar1=gbias[:, 6 + gc:7 + gc])
                            rhn = grupool.tile([128, 512], f32, tag="rhn",
                                               name="rhn")[:, :w]
                            nc.vector.tensor_tensor(
                                out=rhn[:], in0=rz[:, j, :], in1=ph[:], op=OP.mult)
                            nc.vector.tensor_add(out=pi[:], in0=pi[:], in1=rhn[:])
                            nc.scalar.activation(nt[:, j, :], pi[:], AF.Tanh)
                        if s == 0:
                            for m in range(2):
                                d = grupool.tile([128, 2, 512], bf16, tag="d",
                                                 name="d")[:, m, :w]
                                if m == 1:
                                    # h=0: h' = n - z*n
                                    nc.vector.tensor_tensor(
                                        out=d[:], in0=rz[:, 3, :], in1=nt[:, 1, :],
                                        op=OP.mult)
                                    nc.vector.tensor_sub(
                                        out=hT[:, 1, cs], in0=nt[:, 1, :], in1=d[:])
                                    continue
                                nc.vector.tensor_sub(
                                    out=d[:], in0=feat[:, cs], in1=nt[:, 0, :])
                                nc.vector.tensor_tensor(
                                    out=d[:], in0=rz[:, 2, :], in1=d[:], op=OP.mult)
                                nc.vector.tensor_add(
                                    out=hT[:, 0, cs], in0=nt[:, 0, :], in1=d[:])
                            nc.vector.tensor_copy(
                                out=h8[:, :, cs], in_=hT[:, :, cs])
                            return
                        # steps>=1: both m-halves in one [128, 2, w] op each
                        d = grupool.tile([128, 2, 512], bf16, tag="d",
                                         name="d")[:, :, :w]
                        nc.vector.tensor_sub(
                            out=d[:], in0=hT[:, :, cs], in1=nt[:])
                        nc.vector.tensor_tensor(
                            out=d[:], in0=rz[:, 2:4, :], in1=d[:], op=OP.mult)
                        if s < STEPS - 1:
                            # GpSimd deliberately unused: its slow SBUF ops
                            # (~4us per copy) contend for SBUF and slow
                            # concurrent DVE ops 3-6x
                            nc.vector.tensor_add(
                                out=hT[:, :, cs], in0=nt[:], in1=d[:])
                            nc.vector.tensor_add(
                                out=h8[:, :, cs], in0=nt[:], in1=d[:])
                        else:
                            # last step: only the conv head consumes h -> fp8 only
                            nc.vector.tensor_add(
                                out=h8[:, :, cs], in0=nt[:], in1=d[:])

                    # Schedule: within a step, GRU slab k only needs graphs
                    # <= k+1, so its DVE/ACT tail overlaps later graphs'
                    # adjacency mms. Across steps, messages(s+1) for chunks
                    # 0-15 only need h8 cols < 2048 (slabs 0-3), so they run
                    # on the PE while the last slabs' gate chains drain --
                    # removing the step-boundary PE stall.
                    hw_t = [None] * STEPS
                    hw_t[0] = hwpool.tile([128, KCH * T, 256], fp8, tag="hw",
                                          name="hw0")
                    msg_phase(0, hw_t[0], 0, KCH)
                    for s in range(STEPS):
                        hw = hw_t[s]
                        a_phase(0, hw)
                        a_phase(1, hw)
                        gru_slab(s, *SL[0])
                        a_phase(2, hw)
                        gru_slab(s, *SL[1])
                        a_phase(3, hw)
                        gru_slab(s, *SL[2])
                        gru_slab(s, *SL[3])
                        if s < STEPS - 1:
                            hw_t[s + 1] = hwpool.tile(
                                [128, KCH * T, 256], fp8, tag="hw",
                                name=f"hw{s + 1}")
                            msg_phase(s + 1, hw_t[s + 1], 0, KCH - 1)
                        gru_slab(s, *SL[4])
                        if s < STEPS - 1:
                            msg_phase(s + 1, hw_t[s + 1], KCH - 1, KCH)
            # ================= conv heads =================
            # ps_hw pool is still live (outer scope) on banks disjoint from
            # the GGNN gate pools, so conv matmuls don't serialize on the
            # GGNN drain at the phase transition
            if True:

                def conv_stage1(g):
                    base = g * NN
                    # stage 1: all five conv1 output chunks (Y then Z) so PE
                    # has a long uninterrupted run while pools/relu trail
                    y1 = ypool.tile([128, 2, L1], bf16, tag="y1")
                    z1 = zpool.tile([128, 3, L1], bf16, tag="z1")
                    for co in range(2):
                        ps = ps_hw.tile([128, L1], f32, tag="pshw")
                        for k in range(3):
                            nc.tensor.matmul(
                                ps[:], c1w[:, k, :, co, :],
                                h8[:, :, base + k: base + k + L1],
                                start=(k == 0), stop=(k == 2), perf_mode=DR)
                        nc.scalar.activation(
                            y1[:, co, :], ps[:], AF.Relu,
                            bias=cbias[:, co:co + 1] if has_conv_b else 0.0)
                    for co in range(3):
                        ps = ps_hw.tile([128, L1], f32, tag="pshw")
                        for k in range(3):
                            nc.tensor.matmul(
                                ps[:], cc1w[:, k, :, co, :],
                                h8[:, :, base + k: base + k + L1],
                                start=(k == 0), stop=False, perf_mode=DR)
                            nc.tensor.matmul(
                                ps[:], cc1wf[:, k, co, :],
                                feat[:, base + k: base + k + L1],
                                start=False, stop=(k == 2))
                        nc.scalar.activation(
                            z1[:, co, :], ps[:], AF.Relu,
                            bias=cbias[:, 4 + co:5 + co] if has_conv_b else 0.0)
                    # stage 2: pools
                    y1p = ypool.tile([128, 2, P1], bf16, tag="y1p")
                    z1p = zpool.tile([128, 3, P1], bf16, tag="z1p")
                    for co in range(2):
                        nc.vector.tensor_tensor(
                            out=y1p[:, co, :], in0=y1[:, co, 0:510:2],
                            in1=y1[:, co, 1:510:2], op=OP.max)
                        nc.vector.tensor_tensor(
                            out=y1p[:, co, :], in0=y1p[:, co, :],
                            in1=y1[:, co, 2:511:2], op=OP.max)
                    for co in range(3):
                        nc.vector.tensor_tensor(
                            out=z1p[:, co, :], in0=z1[:, co, 0:510:2],
                            in1=z1[:, co, 1:510:2], op=OP.max)
                        nc.vector.tensor_tensor(
                            out=z1p[:, co, :], in0=z1p[:, co, :],
                            in1=z1[:, co, 2:511:2], op=OP.max)
                    return y1p, z1p

                def conv_rest(g, y1p, z1p):
                    # stage 3: second convs (bf16: short free dims are
                    # LDWEIGHTS-bound, DoubleRow wouldn't help)
                    y2 = ypool.tile([128, 2, L2Y], bf16, tag="y2")
                    z2 = zpool.tile([128, 3, L2Z], bf16, tag="z2")
                    for co in range(2):
                        ps = ps_hw.tile([128, L2Y], f32, tag="pshw")
                        mm_acc(nc, ps[:], [
                            (c2w[:, 0, ci, co, :], y1p[:, ci, :]) for ci in range(2)])
                        nc.scalar.activation(
                            y2[:, co, :], ps[:], AF.Relu,
                            bias=cbias[:, 2 + co:3 + co] if has_conv_b else 0.0)
                    for co in range(3):
                        ps = ps_hw.tile([128, L2Z], f32, tag="pshw")
                        mm_acc(nc, ps[:], [
                            (cc2w[:, k, ci, co, :], z1p[:, ci, k:k + L2Z])
                            for k in range(2) for ci in range(3)])
                        nc.scalar.activation(
                            z2[:, co, :], ps[:], AF.Relu,
                            bias=cbias[:, 7 + co:8 + co] if has_conv_b else 0.0)
                    # stage 4: pools + heads
                    y2p = ypool.tile([128, 2, P2], bf16, tag="y2p")
                    z2p = zpool.tile([128, 3, P2], bf16, tag="z2p")
                    for co in range(2):
                        nc.vector.tensor_tensor(
                            out=y2p[:, co, :], in0=y2[:, co, 0:254:2],
                            in1=y2[:, co, 1:254:2], op=OP.max)
                    for co in range(3):
                        nc.vector.tensor_tensor(
                            out=z2p[:, co, :], in0=z2[:, co, 0:254:2],
                            in1=z2[:, co, 1:254:2], op=OP.max)
                    psy = ps_hw.tile([1, P2], f32, tag="pshw")
                    mm_acc(nc, psy[:], [
                        (mlpy[:, co:co + 1], y2p[:, co, :]) for co in range(2)])
                    ys = ypool.tile([1, P2], f32, tag="ys")
                    if has_mlp_b:
                        nc.vector.tensor_scalar_add(
                            out=ys[:], in0=psy[:], scalar1=mbias[:, 0:1])
                    else:
                        nc.vector.tensor_copy(out=ys[:], in_=psy[:])
                    psz = ps_hw.tile([1, P2], f32, tag="pshw")
                    mm_acc(nc, psz[:], [
                        (mlpz[:, co:co + 1], z2p[:, co, :]) for co in range(3)])
                    zs = zpool.tile([1, P2], f32, tag="zs")
                    if has_mlp_b:
                        nc.vector.tensor_scalar_add(
                            out=zs[:], in0=psz[:], scalar1=mbias[:, 1:2])
                    else:
                        nc.vector.tensor_copy(out=zs[:], in_=psz[:])
                    prod = ypool.tile([1, P2], f32, tag="prod")
                    nc.vector.tensor_tensor(
                        out=prod[:], in0=ys[:], in1=zs[:], op=OP.mult)
                    red = ypool.tile([1, 1], f32, tag="red")
                    import concourse.mybir as _mb
                    nc.vector.reduce_sum(red[:], prod[:], axis=_mb.AxisListType.X)
                    nc.scalar.activation(
                        out_sb[:, g:g + 1], red[:], AF.Sigmoid, scale=1.0 / P2)

                # pipeline graphs one deep: stage1(g+1)'s matmuls keep the
                # PE busy while stage3(g) waits on stage-2 pools (DVE)
                pending = conv_stage1(0)
                for g in range(1, GPC):
                    nxt = conv_stage1(g)
                    conv_rest(g - 1, *pending)
                    pending = nxt
                conv_rest(GPC - 1, *pending)

            nc.sync.dma_start(out=d_out[None, :], in_=out_sb[:1, :])

    nc.compile()
    return nc


def _layout_inputs(feature, W_msg, b_msg, gru_w_ih, gru_w_hh, gru_b_ih, gru_b_hh,
                   conv1_w, conv1_b, conv2_w, conv2_b, convc1_w, convc1_b,
                   convc2_w, convc2_b, mlpy_w, mlpy_b, mlpz_w, mlpz_b,
                   edge_src, edge_dst, edge_type):
    """Host-side sharding + SBUF-layout construction. Index math only
    (plus dtype casts / zero padding / transposes of float inputs)."""
    feature = np.asarray(feature, np.float32)
    edge_src = np.asarray(edge_src).astype(np.int64)
    edge_dst = np.asarray(edge_dst).astype(np.int64)
    edge_type = np.asarray(edge_type).astype(np.int64)

    flags = (
        bool(np.any(np.asarray(b_msg))),
        bool(np.any(np.asarray(gru_b_ih)) or np.any(np.asarray(gru_b_hh))),
        bool(np.any(np.asarray(conv1_b)) or np.any(np.asarray(conv2_b))
             or np.any(np.asarray(convc1_b)) or np.any(np.asarray(convc2_b))),
        bool(np.any(np.asarray(mlpy_b)) or np.any(np.asarray(mlpz_b))),
    )
    has_bmsg, has_gru_b, has_conv_b, has_mlp_b = flags

    # ---- adjacency counts, padded to 640/graph ----
    g_of_e = edge_src // NN
    d_loc = edge_dst - g_of_e * NN
    # packed per-core row index of src: 513*(g mod GPC) + s_loc
    src_packed = edge_src - (g_of_e // GPC) * (GPC * NN)
    kc = src_packed // 128           # packed chunk 0..16 (per core)
    pp = src_packed - kc * 128
    kloc = kc - 4 * (g_of_e % GPC)   # strip chunk 0..4
    # A_h[p, core, g, kloc*T+t, d]
    A_h = np.zeros((128, NCORES, GPC, SC, NN), np.float32)
    np.add.at(A_h, (pp, g_of_e // GPC, g_of_e % GPC,
                    kloc * T + edge_type, d_loc), 1.0)

    # ---- shared weight layouts ----
    W_msg = np.asarray(W_msg, np.float32)          # [T, out, in]
    wmsg_l = np.ascontiguousarray(
        W_msg.transpose(2, 0, 1).reshape(2, 128, T, OUT)
        .transpose(1, 0, 2, 3))                     # [p, k, t, o] f32
    wih_l = np.ascontiguousarray(
        np.asarray(gru_w_ih, np.float32).T.reshape(2, 128, 3 * OUT)
        .transpose(1, 0, 2))                        # [p, k, m] f32
    whh_l = np.ascontiguousarray(
        np.asarray(gru_w_hh, np.float32).T.reshape(2, 128, 3 * OUT)
        .transpose(1, 0, 2))

    def conv_lay(w, nci, nco):
        # w: [cout, cin, k] -> [p, k, ci, co, f]; lhsT[cin_part, cout_free]
        w = np.asarray(w, np.float32)
        k = w.shape[2]
        out = np.zeros((128, k, nci, nco, 128), np.float32)
        for kk in range(k):
            wt = w[:, :, kk].T                      # [cin, cout]
            for ci in range(nci):
                for co in range(nco):
                    out[:, kk, ci, co, :] = wt[ci * 128:(ci + 1) * 128,
                                               co * 128:(co + 1) * 128]
        return out

    c1w_l = conv_lay(conv1_w, 2, 2)
    c2w_l = conv_lay(conv2_w, 2, 2).astype(BF16)
    cc1w_l = conv_lay(convc1_w, 3, 3)
    cc2w_l = conv_lay(convc2_w, 3, 3).astype(BF16)
    mlpy_l = np.ascontiguousarray(
        np.asarray(mlpy_w, np.float32).reshape(2, 128).T).astype(BF16)
    mlpz_l = np.ascontiguousarray(
        np.asarray(mlpz_w, np.float32).reshape(3, 128).T).astype(BF16)

    shared = dict(
        wmsg0=np.ascontiguousarray(wmsg_l[:, 0]).astype(BF16),
        wmsg8=np.ascontiguousarray(wmsg_l.reshape(128, 2, T * OUT)).astype(F8),
        wih8=wih_l.astype(F8),
        whh8=whh_l.astype(F8),
        whh0=np.ascontiguousarray(whh_l[:, 0]).astype(BF16),
        c1w=c1w_l.astype(F8),
        c2w=c2w_l,
        cc1w=np.ascontiguousarray(cc1w_l[:, :, 0:2]).astype(F8),
        cc1wf=np.ascontiguousarray(cc1w_l[:, :, 2]).astype(BF16),
        cc2w=cc2w_l, mlpy=mlpy_l, mlpz=mlpz_l)
    if has_bmsg:
        shared["bmsg"] = np.asarray(b_msg, np.float32)
    if has_gru_b:
        gb = np.zeros((128, 12), np.float32)
        gb[:, 0:6] = np.asarray(gru_b_ih, np.float32).reshape(6, 128).T
        gb[:, 6:12] = np.asarray(gru_b_hh, np.float32).reshape(6, 128).T
        shared["gbias"] = gb
    if has_conv_b:
        cb = np.zeros((128, 10), np.float32)
        cb[:, 0:2] = np.asarray(conv1_b, np.float32).reshape(2, 128).T
        cb[:, 2:4] = np.asarray(conv2_b, np.float32).reshape(2, 128).T
        cb[:, 4:7] = np.asarray(convc1_b, np.float32).reshape(3, 128).T
        cb[:, 7:10] = np.asarray(convc2_b, np.float32).reshape(3, 128).T
        shared["cbias"] = cb
    if has_mlp_b:
        shared["mbias"] = np.array(
            [[float(np.asarray(mlpy_b).reshape(-1)[0]),
              float(np.asarray(mlpz_b).reshape(-1)[0])]], np.float32)

    in_maps = []
    for c in range(NCORES):
        g0 = c * GPC
        feat_l = np.zeros((128, NP), np.float32)
        rows = feature[g0 * NN:(g0 + GPC) * NN]                # [2052, 128]
        feat_l[:, :NV] = rows.T
        A_l = np.ascontiguousarray(A_h[:, c]).astype(F8)       # [128,4,20,513]
        m = dict(shared)
        m["feat"] = feat_l.astype(BF16)
        m["adj"] = A_l
        if has_bmsg:
            ind = np.zeros((T, NP), np.float32)
            for g in range(GPC):
                ed_g = (g_of_e // GPC == c) & (g_of_e % GPC == g)
                np.add.at(ind, (edge_type[ed_g], g * NN + d_loc[ed_g]), 1.0)
            m["indeg"] = ind
        in_maps.append(m)
    return flags, in_maps


def kernel(**inputs):
    from concourse.bass_utils import run_bass_kernel_spmd

    flags, in_maps = _layout_inputs(**inputs)
    if flags not in _prog_cache:
        _prog_cache[flags] = _build_program(flags)
    nc = _prog_cache[flags]
    res = run_bass_kernel_spmd(nc, in_maps, core_ids=list(range(NCORES)))
    out = np.concatenate([np.asarray(res.results[c]["out"], np.float32)
                          for c in range(NCORES)])
    return out
